# revision 8
# baseline (speedup 1.0000x reference)
"""GQA causal-attention prefill kernel for 8 TRN2 NeuronCores.

Sharding: tensor-parallel over heads. Core c owns q-heads {2c, 2c+1} and
kv-head c (whole GQA group). Each core computes its heads' attention and a
partial output projection; the host sums the 8 partials (no collectives).

Layout: activations kept transposed [feature, token] on-chip so every matmul
uses natural operand layouts. Matmuls run in float32r (tf32-like, 4x faster
than f32 on the PE at moving-dim >= 256). Softmax skips max-subtraction:
scores are bounded by ||q||*||k||/sqrt(D) <= sqrt(D)*max_gamma^2 ~ 12, safe
in f32 exp.
"""
import os
import sys

for _p in ("/opt/trn_rl_repo", "/root/.axon_site/_ro/trn_rl_repo"):
    if os.path.isdir(_p) and _p not in sys.path:
        sys.path.append(_p)

import numpy as np
import concourse.bacc as bacc
import concourse.mybir as mybir
import concourse.tile as tile
from concourse.bass_utils import run_bass_kernel_spmd

B, S, DIM = 2, 2048, 2048
H, KVH, D = 16, 8, 128
EPS = 1e-6
NCORES = 8
HL = H // NCORES            # q heads per core
SQC = 512                   # sequence chunk (matmul moving dim)
NJ = S // SQC               # chunks per batch
KT = DIM // 128             # contraction tiles for the projections
BS = B * S
SCALE = 1.0 / float(np.sqrt(D))

F32 = mybir.dt.float32
F32R = mybir.dt.float32r
AF = mybir.ActivationFunctionType


def build(debug_dumps=False):
    nc = bacc.Bacc("TRN2", target_bir_lowering=False, debug=False,
                   num_devices=NCORES)
    xt = nc.dram_tensor("xt", [DIM, BS], F32R, kind="ExternalInput").ap()
    wq = nc.dram_tensor("wq", [DIM, HL * D], F32R, kind="ExternalInput").ap()
    wk = nc.dram_tensor("wk", [DIM, D], F32R, kind="ExternalInput").ap()
    wv = nc.dram_tensor("wv", [DIM, D], F32R, kind="ExternalInput").ap()
    wo = nc.dram_tensor("wo", [HL * D, DIM], F32R, kind="ExternalInput").ap()
    gq = nc.dram_tensor("gq", [D, 1], F32, kind="ExternalInput").ap()
    gk = nc.dram_tensor("gk", [D, 1], F32, kind="ExternalInput").ap()
    cost = nc.dram_tensor("cost", [128, S], F32, kind="ExternalInput").ap()
    sint = nc.dram_tensor("sint", [128, S], F32, kind="ExternalInput").ap()
    msk = nc.dram_tensor("msk", [128, 4 * SQC], F32R, kind="ExternalInput").ap()
    onc = nc.dram_tensor("onc", [128, 1], F32R, kind="ExternalInput").ap()
    onr = nc.dram_tensor("onr", [1, 128], F32, kind="ExternalInput").ap()
    idn = nc.dram_tensor("idn", [128, 128], F32R, kind="ExternalInput").ap()
    rmt = nc.dram_tensor("rmt", [128, 128], F32R, kind="ExternalInput").ap()
    out = nc.dram_tensor("out", [DIM, BS], F32, kind="ExternalOutput").ap()
    dbg = {}
    if debug_dumps:
        for nm in ("qt0d", "ktd", "vbd", "ao0d"):
            dbg[nm] = nc.dram_tensor(nm, [128, S], F32R, kind="ExternalOutput").ap()

    with tile.TileContext(nc) as tc:
        with tc.tile_pool(name="const", bufs=1) as cp, \
             tc.tile_pool(name="xp", bufs=6) as xp, \
             tc.tile_pool(name="persist", bufs=1) as pp, \
             tc.tile_pool(name="wrk", bufs=3) as wrk, \
             tc.tile_pool(name="ep", bufs=3) as epool, \
             tc.tile_pool(name="oop", bufs=2) as oop, \
             tc.tile_pool(name="rows", bufs=2) as rp, \
             tc.tile_pool(name="ps_big", bufs=5, space="PSUM") as psb, \
             tc.tile_pool(name="ps_av", bufs=2, space="PSUM") as psa, \
             tc.tile_pool(name="ps_row", bufs=1, space="PSUM") as psr:

            # ---- constants / weights in SBUF ----
            wq_sb = cp.tile([128, KT * HL * D], F32R)
            wk_sb = cp.tile([128, KT * D], F32R)
            wv_sb = cp.tile([128, KT * D], F32R)
            for kt in range(KT):
                nc.sync.dma_start(out=wq_sb[:, kt * HL * D:(kt + 1) * HL * D],
                                  in_=wq[kt * 128:(kt + 1) * 128, :])
                nc.sync.dma_start(out=wk_sb[:, kt * D:(kt + 1) * D],
                                  in_=wk[kt * 128:(kt + 1) * 128, :])
                nc.sync.dma_start(out=wv_sb[:, kt * D:(kt + 1) * D],
                                  in_=wv[kt * 128:(kt + 1) * 128, :])
            wo_sb = [cp.tile([128, DIM], F32R, name=f"wo{h}") for h in range(HL)]
            for h in range(HL):
                nc.sync.dma_start(out=wo_sb[h][:], in_=wo[h * 128:(h + 1) * 128, :])
            cos_sb = cp.tile([128, S], F32)
            sin_sb = cp.tile([128, S], F32)
            nc.sync.dma_start(out=cos_sb[:], in_=cost)
            nc.sync.dma_start(out=sin_sb[:], in_=sint)
            msk_sb = cp.tile([128, 4 * SQC], F32R)
            nc.sync.dma_start(out=msk_sb[:], in_=msk)
            onc_sb = cp.tile([128, 1], F32R)
            nc.sync.dma_start(out=onc_sb[:], in_=onc)
            onr_sb = cp.tile([1, 128], F32)
            nc.sync.dma_start(out=onr_sb[:], in_=onr)
            idn_sb = cp.tile([128, 128], F32R)
            nc.sync.dma_start(out=idn_sb[:], in_=idn)
            rmt_sb = cp.tile([128, 128], F32R)
            nc.sync.dma_start(out=rmt_sb[:], in_=rmt)
            gq_sb = cp.tile([D, 1], F32)
            gk_sb = cp.tile([D, 1], F32)
            nc.sync.dma_start(out=gq_sb[:], in_=gq)
            nc.sync.dma_start(out=gk_sb[:], in_=gk)
            eps_sb = cp.tile([1, 1], F32)
            nc.gpsimd.memset(eps_sb[:], EPS)

            # ---- per-batch persistent buffers (reused across b) ----
            qt_buf = [pp.tile([128, S], F32R, tag=f"qt{h}", name=f"qt{h}") for h in range(HL)]
            kt_buf = pp.tile([128, S], F32R, tag="ktb")
            v_buf = pp.tile([128, S], F32R, tag="vb")
            ao_buf = [pp.tile([128, S], F32R, tag=f"ao{h}", name=f"ao{h}") for h in range(HL)]

            for b in range(B):
                base = b * S
                # ======== phase P: projections + norm + rope ========
                for sc in range(NJ):
                    col = sc * SQC
                    xk = []
                    for kt in range(KT):
                        t = xp.tile([128, SQC], F32R, tag="x")
                        nc.sync.dma_start(
                            out=t[:],
                            in_=xt[kt * 128:(kt + 1) * 128,
                                   base + col:base + col + SQC])
                        xk.append(t)
                    qp = [psb.tile([128, SQC], F32, tag="big", name=f"qp{h}") for h in range(HL)]
                    kp = psb.tile([128, SQC], F32, tag="big")
                    vp = psb.tile([128, SQC], F32, tag="big")
                    for kt in range(KT):
                        st, sp = kt == 0, kt == KT - 1
                        for h in range(HL):
                            nc.tensor.matmul(
                                qp[h][:],
                                wq_sb[:, kt * HL * D + h * D:kt * HL * D + (h + 1) * D],
                                xk[kt][:], start=st, stop=sp)
                        nc.tensor.matmul(kp[:], wk_sb[:, kt * D:(kt + 1) * D],
                                         xk[kt][:], start=st, stop=sp)
                        nc.tensor.matmul(vp[:], wv_sb[:, kt * D:(kt + 1) * D],
                                         xk[kt][:], start=st, stop=sp)

                    # -- rmsnorm + gamma + rope for q heads and k --
                    for ps, g_col, dest in (
                            [(qp[h], gq_sb, qt_buf[h]) for h in range(HL)]
                            + [(kp, gk_sb, kt_buf)]):
                        sqr = wrk.tile([128, SQC], F32R, tag="sqr")
                        nc.scalar.activation(sqr[:], ps[:], AF.Square)
                        tsb = wrk.tile([128, SQC], F32R, tag="tsb")
                        nc.scalar.activation(tsb[:], ps[:], AF.Copy,
                                             scale=g_col[:])
                        ssum = psr.tile([1, SQC], F32, tag="row")
                        nc.tensor.matmul(ssum[:], onc_sb[:], sqr[:],
                                         start=True, stop=True)
                        sdn = rp.tile([1, SQC], F32, tag="sdn")
                        nc.scalar.activation(sdn[:], ssum[:], AF.Sqrt,
                                             scale=1.0 / D, bias=eps_sb[:])
                        rs = rp.tile([1, SQC], F32, tag="rs")
                        nc.vector.reciprocal(rs[:], sdn[:])
                        bc = psb.tile([128, SQC], F32, tag="big")
                        nc.tensor.matmul(bc[:], onr_sb[:], rs[:],
                                         start=True, stop=True)
                        c_sl = cos_sb[:, col:col + SQC]
                        s_sl = sin_sb[:, col:col + SQC]
                        rot = psb.tile([128, SQC], F32, tag="big")
                        nc.tensor.matmul(rot[:], rmt_sb[:], tsb[:],
                                         start=True, stop=True)
                        m1 = wrk.tile([128, SQC], F32, tag="m1")
                        m2 = wrk.tile([128, SQC], F32, tag="m2")
                        nc.vector.tensor_mul(m1[:], tsb[:], c_sl)
                        nc.vector.tensor_mul(m2[:], rot[:], s_sl)
                        u = wrk.tile([128, SQC], F32, tag="m1")
                        nc.vector.tensor_add(u[:], m1[:], m2[:])
                        nc.vector.tensor_mul(dest[:, col:col + SQC],
                                             u[:], bc[:])

                    # -- V: copy + transpose to natural [s, d] layout --
                    vt = wrk.tile([128, SQC], F32R, tag="tsb")
                    nc.scalar.activation(vt[:], vp[:], AF.Copy)
                    for cq in range(4):
                        vq = psb.tile([128, 128], F32R, tag="big")
                        nc.tensor.transpose(vq[:], vt[:, cq * 128:(cq + 1) * 128],
                                            idn_sb[:])
                        ti = sc * 4 + cq
                        nc.vector.tensor_copy(
                            v_buf[:, ti * 128:(ti + 1) * 128], vq[:])

                # ======== phase A: attention ========
                for h in range(HL):
                    for j in range(NJ):
                        nsk = 4 * j + 4
                        avp = psa.tile([128, SQC], F32, tag="av")
                        dnp = psr.tile([1, SQC], F32, tag="row")
                        for i in range(nsk):
                            scp = psb.tile([128, SQC], F32, tag="big")
                            nc.tensor.matmul(
                                scp[:], kt_buf[:, i * 128:(i + 1) * 128],
                                qt_buf[h][:, j * SQC:(j + 1) * SQC],
                                start=True, stop=True)
                            e = epool.tile([128, SQC], F32R, tag="e")
                            nc.scalar.activation(e[:], scp[:], AF.Exp,
                                                 scale=SCALE)
                            if i >= 4 * j:
                                m = i - 4 * j
                                em = epool.tile([128, SQC], F32R, tag="e")
                                nc.vector.tensor_mul(
                                    em[:], e[:],
                                    msk_sb[:, m * SQC:(m + 1) * SQC])
                                e = em
                            st, sp = i == 0, i == nsk - 1
                            nc.tensor.matmul(avp[:],
                                             v_buf[:, i * 128:(i + 1) * 128],
                                             e[:], start=st, stop=sp)
                            nc.tensor.matmul(dnp[:], onc_sb[:], e[:],
                                             start=st, stop=sp)
                        rcp = rp.tile([1, SQC], F32, tag="sdn")
                        nc.vector.reciprocal(rcp[:], dnp[:])
                        bcd = psb.tile([128, SQC], F32, tag="big")
                        nc.tensor.matmul(bcd[:], onr_sb[:], rcp[:],
                                         start=True, stop=True)
                        avs = wrk.tile([128, SQC], F32, tag="sqr")
                        nc.scalar.activation(avs[:], avp[:], AF.Copy)
                        nc.vector.tensor_mul(
                            ao_buf[h][:, j * SQC:(j + 1) * SQC],
                            avs[:], bcd[:])

                if debug_dumps and b == 0:
                    nc.sync.dma_start(out=dbg["qt0d"], in_=qt_buf[0][:])
                    nc.sync.dma_start(out=dbg["ktd"], in_=kt_buf[:])
                    nc.sync.dma_start(out=dbg["vbd"], in_=v_buf[:])
                    nc.sync.dma_start(out=dbg["ao0d"], in_=ao_buf[0][:])

                # ======== phase O: output projection (partial) ========
                for j in range(NJ):
                    for dt in range(KT):
                        op = psb.tile([128, SQC], F32, tag="big")
                        for h in range(HL):
                            nc.tensor.matmul(
                                op[:], wo_sb[h][:, dt * 128:(dt + 1) * 128],
                                ao_buf[h][:, j * SQC:(j + 1) * SQC],
                                start=(h == 0), stop=(h == HL - 1))
                        oo = oop.tile([128, SQC], F32, tag="oo")
                        nc.vector.tensor_copy(oo[:], op[:])
                        nc.sync.dma_start(
                            out=out[dt * 128:(dt + 1) * 128,
                                    base + j * SQC:base + (j + 1) * SQC],
                            in_=oo[:])
    nc.compile()
    return nc


_NC_CACHE = None


def _get_nc():
    global _NC_CACHE
    if _NC_CACHE is None:
        _NC_CACHE = build()
    return _NC_CACHE


def kernel(x, wq, wk, wv, wo, q_gamma, k_gamma, cos_cache, sin_cache):
    x = np.asarray(x, dtype=np.float32)
    wq = np.asarray(wq, dtype=np.float32)
    wk = np.asarray(wk, dtype=np.float32)
    wv = np.asarray(wv, dtype=np.float32)
    wo = np.asarray(wo, dtype=np.float32)
    q_gamma = np.asarray(q_gamma, dtype=np.float32)
    k_gamma = np.asarray(k_gamma, dtype=np.float32)
    cos_cache = np.asarray(cos_cache, dtype=np.float32)
    sin_cache = np.asarray(sin_cache, dtype=np.float32)

    xt = np.ascontiguousarray(x.reshape(BS, DIM).T)
    cos_t = cos_cache[:S].T
    sin_t = sin_cache[:S].T
    cost = np.ascontiguousarray(np.concatenate([cos_t, cos_t], axis=0))
    sint = np.ascontiguousarray(np.concatenate([sin_t, sin_t], axis=0))
    gq = np.ascontiguousarray(q_gamma[:, None])
    gk = np.ascontiguousarray(k_gamma[:, None])
    p = np.arange(128)[:, None]
    c = np.arange(SQC)[None, :]
    msk = np.concatenate(
        [(p + 128 * m <= c).astype(np.float32) for m in range(4)], axis=1)
    onc = np.ones((128, 1), np.float32)
    onr = np.ones((1, 128), np.float32)
    idn = np.eye(128, dtype=np.float32)
    pmat = np.zeros((128, 128), np.float32)
    pmat[np.arange(64), np.arange(64) + 64] = -1.0
    pmat[np.arange(64) + 64, np.arange(64)] = 1.0
    rmt = np.ascontiguousarray(pmat.T)

    in_maps = []
    for cid in range(NCORES):
        in_maps.append({
            "xt": xt,
            "wq": np.ascontiguousarray(wq[:, cid * HL * D:(cid + 1) * HL * D]),
            "wk": np.ascontiguousarray(wk[:, cid * D:(cid + 1) * D]),
            "wv": np.ascontiguousarray(wv[:, cid * D:(cid + 1) * D]),
            "wo": np.ascontiguousarray(wo[cid * HL * D:(cid + 1) * HL * D, :]),
            "gq": gq, "gk": gk, "cost": cost, "sint": sint,
            "msk": msk, "onc": onc, "onr": onr, "idn": idn, "rmt": rmt,
        })

    nc = _get_nc()
    trace = os.environ.get("KERNEL_TRACE") == "1"
    r = run_bass_kernel_spmd(nc, in_maps, core_ids=list(range(NCORES)),
                             trace=trace)
    if trace:
        kernel.last_exec_time_ns = r.exec_time_ns
    acc = np.zeros((DIM, BS), np.float64)
    for cid in range(NCORES):
        acc += r.results[cid]["out"]
    return np.ascontiguousarray(
        acc.T.reshape(B, S, DIM).astype(np.float32))
